# revision 23
# baseline (speedup 1.0000x reference)
"""Trainium2 Bass kernel for: out = conv3x3(x, weight*A_w) * sigmoid(conv3x3(relu(conv3x3(x, se_w1)), se_w2))

Sharding: data-parallel over batch B=8 -> 8 NeuronCores (one image per core);
weight / A_w / se_w1 / se_w2 replicated to every core. The conv weights are
passed transposed to [ci, kh, kw, co] (host-side layout prep during sharding)
so the matmul stationary operand loads straight from DRAM.

Per-core kernel (direct conv as implicit GEMM on the TensorEngine):
  - x stored column-padded [ci, 56, 58] bf16 in SBUF (pad cols zeroed,
    +1-element guards at both flat ends) so every 3x3 tap is a contiguous
    1-D shifted window (the matmul ISA requires single-free-dim operands).
  - row taps at the image top/bottom use clipped row ranges; the center tap
    is issued first per ci-block pass (full coverage, start=True), the
    clipped taps accumulate -> exact zero-padding semantics.
  - A_w applied on-device as a VectorE broadcast multiply during the
    f32 -> bf16 weight cast.
  - compute dtype bf16 (fp32 PSUM accumulate), rel-err vs fp32 ~3e-3.
  - thin SE-branch matmul groups (16-wide) are interleaved with dense
    128x128 main-conv groups to keep the PE activity monitor from
    re-throttling the clock (HAM).
  - main-conv PSUM tiles drain to SBUF; the attention multiply is fused
    when `a` for that tile is already available, otherwise applied in a
    deferred VectorE pass before the output DMA.
"""

import numpy as np

import concourse.bass as bass  # noqa: F401
import concourse.mybir as mybir
import concourse.tile as tile
from concourse import bacc
from concourse.bass_utils import run_bass_kernel_spmd
from concourse.masks import make_identity

B, C, H, W = 8, 256, 56, 56
HW = H * W
WP = W + 2                      # padded row width (c=0 left pad, c=57 right pad)
HWP = H * WP                    # 3248
CMID = 16
N_CORES = 8
RT = 8                          # output rows per PSUM tile
NT = H // RT                    # 7
F32 = mybir.dt.float32
BF16 = mybir.dt.bfloat16

# center tap first within each ci-block pass
TAPS = [(0, 0)] + [
    (dh, dw) for dh in (-1, 0, 1) for dw in (-1, 0, 1) if (dh, dw) != (0, 0)
]


def _rows(r0, dh):
    """Clipped local row range [rl, rh) of a tile at base row r0 for row-tap dh."""
    return max(0, -dh - r0), min(RT, H - dh - r0)


def build():
    nc = bacc.Bacc("TRN2", target_bir_lowering=False, debug=False, num_devices=N_CORES)

    x_d = nc.dram_tensor("x", [C, H, W], F32, kind="ExternalInput").ap()
    # transposed on host: [ci, kh, kw, co]
    wt_d = nc.dram_tensor("weightT", [C, 3, 3, C], F32, kind="ExternalInput").ap()
    aw_d = nc.dram_tensor("A_w", [1, C, 3, 3], F32, kind="ExternalInput").ap()
    # transposed on host: [ci, kh, kw, cmid]
    w1t_d = nc.dram_tensor("se_w1T", [C, 3, 3, CMID], F32, kind="ExternalInput").ap()
    w2_d = nc.dram_tensor("se_w2", [1, CMID, 3, 3], F32, kind="ExternalInput").ap()
    out_d = nc.dram_tensor("out", [C, H, W], F32, kind="ExternalOutput").ap()

    x_v = x_d.rearrange("(b p) h w -> b p (h w)", b=2)                  # [2,128,3136]
    wt_v = wt_d.rearrange("(b p) kh kw co -> b p (kh kw co)", b=2)      # [2,128,2304]
    aw_v = aw_d[0].rearrange("(b p) kh kw -> b p (kh kw)", b=2)         # [2,128,9]
    w1t_v = w1t_d.rearrange("(b p) kh kw co -> b p (kh kw co)", b=2)    # [2,128,144]
    w2_v = w2_d[0].rearrange("p kh kw -> p (kh kw)")                    # [16,9]
    out_v = out_d.rearrange("(b p) h w -> b p h w", b=2)                # [2,128,56,56]

    with tile.TileContext(nc) as tc:
        with (
            tc.tile_pool(name="sb", bufs=1) as sb,
            tc.tile_pool(name="ps", space="PSUM", bufs=2) as ps,
        ):
            # +2: one guard element at each flat end (dw=+-1 at image corners)
            xs = [sb.tile([128, HWP + 2], BF16, name=f"xs{i}") for i in range(2)]
            xstage = [sb.tile([128, HW], F32, name=f"xstage{i}") for i in range(2)]
            wrt = [sb.tile([128, 2304], F32, name=f"wrt{i}") for i in range(2)]
            w1rt = [sb.tile([128, 9 * CMID], F32, name=f"w1rt{i}") for i in range(2)]
            aw = [sb.tile([128, 9], F32, name=f"aw{i}") for i in range(2)]
            w2s = sb.tile([CMID, 9], F32, name="w2s")
            wmod = [sb.tile([128, 9 * 256], BF16, name=f"wmod{i}") for i in range(2)]
            mid = sb.tile([CMID, HWP + 2], BF16, name="mid")
            identE = sb.tile([96, CMID], BF16, name="identE")
            identTE = sb.tile([96, 128], BF16, name="identTE")
            u1pp = [sb.tile([96, RT * WP], BF16, name=f"u1pp{k}") for k in range(2)]
            u2pp = [sb.tile([96, RT * WP], BF16, name=f"u2pp{k}") for k in range(2)]
            w2pack = sb.tile([CMID, 3 * 96], BF16, name="w2pack")
            w1pack = [sb.tile([128, 3 * 96], BF16, name=f"w1pack{i}") for i in range(2)]
            asb = sb.tile([128, HWP], F32, name="asb")
            osb = [sb.tile([128, HWP], F32, name=f"osb{c}") for c in range(2)]

            # -------- loads --------
            # x first (the PE's first dependency), chunked so the bf16 cast
            # pipelines behind the DMA; one ci-block per HWDGE queue.
            HHALF = H // 2
            for i in range(2):
                nc.sync.dma_start(wrt[i], wt_v[i])
            for i in range(2):
                q = nc.scalar if i == 0 else nc.sync
                for h0 in (0, HHALF):
                    q.dma_start(
                        xstage[i][:, h0 * W : (h0 + HHALF) * W],
                        x_v[i][:, h0 * W : (h0 + HHALF) * W],
                    )
            for i in range(2):
                nc.gpsimd.dma_start(w1rt[i], w1t_v[i])
                nc.gpsimd.dma_start(aw[i], aw_v[i])
            nc.gpsimd.dma_start(w2s, w2_v)

            # -------- x pad + cast (DVE, ahead of weight prep) --------
            for tl, np_ in ((xs[0], 128), (xs[1], 128), (mid, CMID)):
                nc.vector.memset(tl[:np_, 0:2], 0.0)
                nc.vector.memset(tl[:np_, HWP : HWP + 2], 0.0)
                pads = tl[:np_, 1 + W + 1 : 1 + W + 1 + (H - 1) * WP].rearrange(
                    "p (h c) -> p h c", c=WP
                )
                nc.vector.memset(pads[:, :, 0:2], 0.0)
            for h0 in (0, HHALF):
                for i in range(2):
                    xsv = xs[i][:, 1 : 1 + HWP].rearrange("p (h c) -> p h c", c=WP)
                    nc.vector.tensor_copy(
                        xsv[:, h0 : h0 + HHALF, 1 : W + 1],
                        xstage[i][:, h0 * W : (h0 + HHALF) * W].rearrange(
                            "p (h w) -> p h w", w=W
                        ),
                    )

            # -------- weight prep (VectorE only, no PE) --------
            for i in range(2):
                # wmod[ci, k, co] = weightT[ci, k, co] * A_w[ci, k]  (cast to bf16)
                nc.vector.tensor_mul(
                    wmod[i].rearrange("p (k co) -> p k co", co=256),
                    wrt[i].rearrange("p (k co) -> p k co", co=256),
                    aw[i].unsqueeze(2).broadcast_to([128, 9, 256]),
                )
            for k in range(2):
                nc.vector.memset(u1pp[k], 0.0)
                nc.vector.memset(u2pp[k], 0.0)
            # identity selectors, one copy per 32-aligned strip (matmul
            # operands must share a 32-aligned partition base)
            nc.vector.memset(identE, 0.0)
            nc.vector.memset(identTE, 0.0)
            for g in range(3):
                make_identity(nc, identE[32 * g : 32 * g + CMID, :], nomemset=True)
                nc.vector.tensor_copy(
                    identTE[32 * g : 32 * g + CMID, :].rearrange(
                        "p (r c) -> p r c", c=CMID
                    ),
                    identE[32 * g : 32 * g + CMID, :]
                    .unsqueeze(1)
                    .broadcast_to([CMID, 8, CMID]),
                )
            # kw groups packed into stationary cols at 32-col strides
            for i in range(2):
                nc.vector.memset(w1pack[i], 0.0)
                nc.vector.tensor_copy(
                    w1pack[i]
                    .rearrange("p (kh kw co) -> p kh kw co", kh=3, kw=3)[
                        :, :, :, :CMID
                    ],
                    w1rt[i].rearrange("p (kh kw co) -> p kh kw co", kh=3, kw=3),
                )
            nc.vector.memset(w2pack, 0.0)
            nc.vector.tensor_copy(
                w2pack.rearrange("p (kh kw co) -> p kh kw co", kh=3, kw=3)[
                    :, :, :, :CMID
                ],
                w2s.rearrange("p (kh kw) -> p kh kw", kh=3)
                .unsqueeze(3)
                .broadcast_to([CMID, 3, 3, CMID]),
            )

            mid_v = mid[:, 1 : 1 + HWP].rearrange("p (h c) -> p h c", c=WP)
            TFv = RT * WP
            wmod_v = [wmod[i].rearrange("p (k co) -> p k co", co=256) for i in range(2)]

            # -------- conv group emitters --------
            # SE convs: the 3 kw taps are packed into the stationary columns
            # (48 = 3 kw x 16 ch), then reduced across partition groups with
            # +-1-shifted identity matmuls. Junk in pad columns only.
            def conv1_pack(t):
                r0 = t * RT
                mps = ps.tile([96, TFv], F32, name="mps96", tag="pack", bufs=2)
                n_mm = 0
                for i in range(2):
                    for dh in (0, -1, 1):
                        kh = dh + 1
                        rl, rh = _rows(r0, dh)
                        n_mm += 1
                        nc.tensor.matmul(
                            mps[:, rl * WP : rh * WP],
                            w1pack[i][:, kh * 96 : (kh + 1) * 96],
                            xs[i][:, 1 + (r0 + rl + dh) * WP :][:128, : (rh - rl) * WP],
                            start=(n_mm == 1),
                            stop=(n_mm == 6),
                        )
                u = u1pp[t % 2]
                # drain each kw strip with its +-1 column shift baked in, so
                # one K=96 selector matmul can reduce without further shifts
                ident = mybir.ActivationFunctionType.Identity
                nc.vector.tensor_copy(u[0:16, 1:TFv], mps[0:16, 0 : TFv - 1])
                nc.scalar.activation(u[32:48, :], mps[32:48, :], ident)
                nc.scalar.activation(u[64:80, 0 : TFv - 1], mps[64:80, 1:TFv], ident)
                return u

            def conv1_sel(t, u):
                r0 = t * RT
                mid_ps = ps.tile([CMID, TFv], F32, name="mid_ps", tag="red", bufs=2)
                nc.tensor.matmul(mid_ps, identE, u, start=True, stop=True)
                mpv = mid_ps.rearrange("p (h c) -> p h c", c=WP)
                nc.scalar.activation(
                    mid_v[:, r0 : r0 + RT, 1 : W + 1],
                    mpv[:, :, 1 : W + 1],
                    mybir.ActivationFunctionType.Relu,
                )

            def conv2_pack(t):
                r0 = t * RT
                ups = ps.tile([96, TFv], F32, name="u2ps", tag="pack", bufs=2)
                n_mm = 0
                for dh in (0, -1, 1):
                    kh = dh + 1
                    rl, rh = _rows(r0, dh)
                    n_mm += 1
                    nc.tensor.matmul(
                        ups[:, rl * WP : rh * WP],
                        w2pack[:, kh * 96 : (kh + 1) * 96],
                        mid[:, 1 + (r0 + rl + dh) * WP :][:CMID, : (rh - rl) * WP],
                        start=(n_mm == 1),
                        stop=(n_mm == 3),
                    )
                u = u2pp[t % 2]
                ident = mybir.ActivationFunctionType.Identity
                nc.vector.tensor_copy(u[0:16, 1:TFv], ups[0:16, 0 : TFv - 1])
                nc.scalar.activation(u[32:48, :], ups[32:48, :], ident)
                nc.scalar.activation(u[64:80, 0 : TFv - 1], ups[64:80, 1:TFv], ident)
                return u

            def conv2_sel(t, u):
                r0 = t * RT
                aps = ps.tile([128, TFv], F32, name="aps", tag="red", bufs=2)
                nc.tensor.matmul(aps, identTE, u, start=True, stop=True)
                nc.scalar.activation(
                    asb[:, r0 * WP : (r0 + RT) * WP],
                    aps,
                    mybir.ActivationFunctionType.Sigmoid,
                )

            def main_group(t, c, fused):
                r0 = t * RT
                yps = ps.tile([128, RT * WP], F32, name="yps", tag="yps", bufs=4)
                n_mm = 0
                for i in range(2):
                    for dh, dw in TAPS:
                        k = (dh + 1) * 3 + (dw + 1)
                        rl, rh = _rows(r0, dh)
                        n_mm += 1
                        nc.tensor.matmul(
                            yps[:, rl * WP : rh * WP],
                            wmod_v[i][:, k, c * 128 : (c + 1) * 128],
                            xs[i][:, 1 + (r0 + rl + dh) * WP + dw :][:128, : (rh - rl) * WP],
                            start=(n_mm == 1),
                            stop=(n_mm == 18),
                        )
                dst = osb[c][:, r0 * WP : (r0 + RT) * WP]
                if fused:
                    nc.vector.tensor_mul(dst, yps, asb[:, r0 * WP : (r0 + RT) * WP])
                    ov = osb[c].rearrange("p (h c) -> p h c", c=WP)
                    q = nc.sync if (t + c) % 2 == 0 else nc.scalar
                    q.dma_start(
                        out_v[c][:, r0 : r0 + RT, :], ov[:, r0 : r0 + RT, 1 : W + 1]
                    )
                else:
                    nc.vector.tensor_copy(dst, yps)

            # -------- interleaved schedule --------
            # main groups in issue order; SE groups threaded between them so
            # the PE never sees a long run of thin (16-wide) matmuls.
            main_q = [(t, c) for t in range(NT) for c in range(2)]
            mq = iter(main_q)
            deferred = []
            sig_done = [False] * NT

            def emit_main(n, fused_allowed):
                for _ in range(n):
                    tc_ = next(mq, None)
                    if tc_ is None:
                        return
                    t, c = tc_
                    if sig_done[t] and fused_allowed:
                        main_group(t, c, fused=True)
                    else:
                        main_group(t, c, fused=False)
                        deferred.append((t, c))

            def flush_deferred():
                rest = []
                for t, c in deferred:
                    if not sig_done[t]:
                        rest.append((t, c))
                        continue
                    r0 = t * RT
                    dst = osb[c][:, r0 * WP : (r0 + RT) * WP]
                    nc.vector.tensor_mul(dst, dst, asb[:, r0 * WP : (r0 + RT) * WP])
                    ov = osb[c].rearrange("p (h c) -> p h c", c=WP)
                    q = nc.sync if (t + c) % 2 == 0 else nc.scalar
                    q.dma_start(
                        out_v[c][:, r0 : r0 + RT, :], ov[:, r0 : r0 + RT, 1 : W + 1]
                    )
                deferred[:] = rest

            u_prev = None
            for t in range(NT):
                u = conv1_pack(t)
                if u_prev is not None:
                    conv1_sel(t - 1, u_prev)
                    emit_main(1, fused_allowed=False)
                u_prev = u
            conv1_sel(NT - 1, u_prev)
            emit_main(1, fused_allowed=False)
            u_prev = None
            for t in range(NT):
                u = conv2_pack(t)
                if u_prev is not None:
                    conv2_sel(t - 1, u_prev)
                    sig_done[t - 1] = True
                    emit_main(1, fused_allowed=True)
                    flush_deferred()
                u_prev = u
            conv2_sel(NT - 1, u_prev)
            sig_done[NT - 1] = True
            # remaining main groups: `a` is fully available, fuse the multiply
            emit_main(len(main_q), fused_allowed=True)
            flush_deferred()

    nc.compile()
    return nc


_NC = None


def make_in_maps(x, weight, A_w, se_w1, se_w2):
    x = np.ascontiguousarray(np.asarray(x, dtype=np.float32))
    weightT = np.ascontiguousarray(
        np.asarray(weight, dtype=np.float32).transpose(1, 2, 3, 0)
    )
    A_w = np.ascontiguousarray(np.asarray(A_w, dtype=np.float32))
    se_w1T = np.ascontiguousarray(
        np.asarray(se_w1, dtype=np.float32).transpose(1, 2, 3, 0)
    )
    se_w2 = np.ascontiguousarray(np.asarray(se_w2, dtype=np.float32))

    in_maps = [
        {
            "x": np.ascontiguousarray(x[b]),
            "weightT": weightT,
            "A_w": A_w,
            "se_w1T": se_w1T,
            "se_w2": se_w2,
        }
        for b in range(B)
    ]
    return in_maps


def kernel(x, weight, A_w, se_w1, se_w2):
    global _NC
    if _NC is None:
        _NC = build()
    in_maps = make_in_maps(x, weight, A_w, se_w1, se_w2)
    res = run_bass_kernel_spmd(_NC, in_maps, list(range(N_CORES)))
    out = np.stack([res.results[b]["out"] for b in range(B)], axis=0)
    return out


# revision 24
# speedup vs baseline: 1.1318x; 1.1318x over previous
"""Trainium2 Bass kernel for: out = conv3x3(x, weight*A_w) * sigmoid(conv3x3(relu(conv3x3(x, se_w1)), se_w2))

Sharding: data-parallel over batch B=8 -> 8 NeuronCores (one image per core);
weight / A_w / se_w1 / se_w2 replicated to every core. The conv weights are
passed transposed to [ci, kh, kw, co] (host-side layout prep during sharding)
so the matmul stationary operand loads straight from DRAM.

Per-core kernel (direct conv as implicit GEMM on the TensorEngine):
  - x stored column-padded [ci, 56, 58] bf16 in SBUF (pad cols zeroed,
    +1-element guards at both flat ends) so every 3x3 tap is a contiguous
    1-D shifted window (the matmul ISA requires single-free-dim operands).
  - row taps at the image top/bottom use clipped row ranges; the center tap
    is issued first per ci-block pass (full coverage, start=True), the
    clipped taps accumulate -> exact zero-padding semantics.
  - A_w applied on-device as a VectorE broadcast multiply during the
    f32 -> bf16 weight cast.
  - compute dtype bf16 (fp32 PSUM accumulate), rel-err vs fp32 ~3e-3.
  - thin SE-branch matmul groups (16-wide) are interleaved with dense
    128x128 main-conv groups to keep the PE activity monitor from
    re-throttling the clock (HAM).
  - main-conv PSUM tiles drain to SBUF; the attention multiply is fused
    when `a` for that tile is already available, otherwise applied in a
    deferred VectorE pass before the output DMA.
"""

import numpy as np

import concourse.bass as bass  # noqa: F401
import concourse.mybir as mybir
import concourse.tile as tile
from concourse import bacc
from concourse.bass_utils import run_bass_kernel_spmd
from concourse.masks import make_identity

B, C, H, W = 8, 256, 56, 56
HW = H * W
WP = W + 2                      # padded row width (c=0 left pad, c=57 right pad)
HWP = H * WP                    # 3248
CMID = 16
N_CORES = 8
RT = 8                          # output rows per PSUM tile
NT = H // RT                    # 7
F32 = mybir.dt.float32
BF16 = mybir.dt.bfloat16

# center tap first within each ci-block pass
TAPS = [(0, 0)] + [
    (dh, dw) for dh in (-1, 0, 1) for dw in (-1, 0, 1) if (dh, dw) != (0, 0)
]


def _rows(r0, dh):
    """Clipped local row range [rl, rh) of a tile at base row r0 for row-tap dh."""
    return max(0, -dh - r0), min(RT, H - dh - r0)


def build():
    nc = bacc.Bacc("TRN2", target_bir_lowering=False, debug=False, num_devices=N_CORES)

    x_d = nc.dram_tensor("x", [C, H, W], F32, kind="ExternalInput").ap()
    # transposed on host: [ci, kh, kw, co]
    wt_d = nc.dram_tensor("weightT", [C, 3, 3, C], F32, kind="ExternalInput").ap()
    aw_d = nc.dram_tensor("A_w", [1, C, 3, 3], F32, kind="ExternalInput").ap()
    # transposed on host: [ci, kh, kw, cmid]
    w1t_d = nc.dram_tensor("se_w1T", [C, 3, 3, CMID], F32, kind="ExternalInput").ap()
    w2_d = nc.dram_tensor("se_w2", [1, CMID, 3, 3], F32, kind="ExternalInput").ap()
    out_d = nc.dram_tensor("out", [C, H, W], F32, kind="ExternalOutput").ap()

    x_v = x_d.rearrange("(b p) h w -> b p (h w)", b=2)                  # [2,128,3136]
    wt_v = wt_d.rearrange("(b p) kh kw co -> b p (kh kw co)", b=2)      # [2,128,2304]
    aw_v = aw_d[0].rearrange("(b p) kh kw -> b p (kh kw)", b=2)         # [2,128,9]
    w1t_v = w1t_d.rearrange("(b p) kh kw co -> b p (kh kw co)", b=2)    # [2,128,144]
    w2_v = w2_d[0].rearrange("p kh kw -> p (kh kw)")                    # [16,9]
    out_v = out_d.rearrange("(b p) h w -> b p h w", b=2)                # [2,128,56,56]

    with tile.TileContext(nc) as tc:
        with (
            tc.tile_pool(name="sb", bufs=1) as sb,
            tc.tile_pool(name="ps", space="PSUM", bufs=2) as ps,
        ):
            # +2: one guard element at each flat end (dw=+-1 at image corners)
            xs = [sb.tile([128, HWP + 2], BF16, name=f"xs{i}") for i in range(2)]
            xstage = [
                [sb.tile([128, HW // 2], F32, name=f"xstage{i}{k}") for k in range(2)]
                for i in range(2)
            ]
            wrt = [sb.tile([128, 2304], F32, name=f"wrt{i}") for i in range(2)]
            w1rt = [sb.tile([128, 9 * CMID], F32, name=f"w1rt{i}") for i in range(2)]
            aw = [sb.tile([128, 9], F32, name=f"aw{i}") for i in range(2)]
            w2s = sb.tile([CMID, 9], F32, name="w2s")
            wmod = [sb.tile([128, 9 * 256], BF16, name=f"wmod{i}") for i in range(2)]
            mid = sb.tile([CMID, HWP + 2], BF16, name="mid")
            identE = sb.tile([96, CMID], BF16, name="identE")
            identTE = sb.tile([96, 128], BF16, name="identTE")
            u1pp = [sb.tile([96, RT * WP], BF16, name=f"u1pp{k}") for k in range(2)]
            u2pp = [sb.tile([96, RT * WP], BF16, name=f"u2pp{k}") for k in range(2)]
            w2pack = sb.tile([CMID, 3 * 96], BF16, name="w2pack")
            w1pack = [sb.tile([128, 3 * 96], BF16, name=f"w1pack{i}") for i in range(2)]
            asb = sb.tile([128, HWP], F32, name="asb")
            osb = [sb.tile([128, HWP], F32, name=f"osb{c}") for c in range(2)]

            # -------- loads --------
            # x first (the PE's first dependency), chunked so the bf16 cast
            # pipelines behind the DMA; one ci-block per HWDGE queue.
            HHALF = H // 2
            for i in range(2):
                nc.sync.dma_start(wrt[i], wt_v[i])
            for i in range(2):
                q = nc.scalar if i == 0 else nc.sync
                for k, h0 in enumerate((0, HHALF)):
                    q.dma_start(
                        xstage[i][k],
                        x_v[i][:, h0 * W : (h0 + HHALF) * W],
                    )
            for i in range(2):
                nc.gpsimd.dma_start(w1rt[i], w1t_v[i])
                nc.gpsimd.dma_start(aw[i], aw_v[i])
            nc.gpsimd.dma_start(w2s, w2_v)

            # -------- x pad + cast (DVE, ahead of weight prep) --------
            for tl, np_ in ((xs[0], 128), (xs[1], 128), (mid, CMID)):
                nc.vector.memset(tl[:np_, 0:2], 0.0)
                nc.vector.memset(tl[:np_, HWP : HWP + 2], 0.0)
                pads = tl[:np_, 1 + W + 1 : 1 + W + 1 + (H - 1) * WP].rearrange(
                    "p (h c) -> p h c", c=WP
                )
                nc.vector.memset(pads[:, :, 0:2], 0.0)
            def emit_casts():
                for k, h0 in enumerate((0, HHALF)):
                    for i in range(2):
                        xsv = xs[i][:, 1 : 1 + HWP].rearrange("p (h c) -> p h c", c=WP)
                        nc.vector.tensor_copy(
                            xsv[:, h0 : h0 + HHALF, 1 : W + 1],
                            xstage[i][k].rearrange("p (h w) -> p h w", w=W),
                        )

            # -------- weight prep (VectorE only, no PE) --------
            for k in range(2):
                nc.vector.memset(u1pp[k], 0.0)
                nc.vector.memset(u2pp[k], 0.0)
            # identity selectors, one copy per 32-aligned strip (matmul
            # operands must share a 32-aligned partition base)
            nc.vector.memset(identE, 0.0)
            nc.vector.memset(identTE, 0.0)
            for g in range(3):
                make_identity(nc, identE[32 * g : 32 * g + CMID, :], nomemset=True)
                nc.vector.tensor_copy(
                    identTE[32 * g : 32 * g + CMID, :].rearrange(
                        "p (r c) -> p r c", c=CMID
                    ),
                    identE[32 * g : 32 * g + CMID, :]
                    .unsqueeze(1)
                    .broadcast_to([CMID, 8, CMID]),
                )
            # kw groups packed into stationary cols at 32-col strides
            for i in range(2):
                nc.vector.memset(w1pack[i], 0.0)
                nc.vector.tensor_copy(
                    w1pack[i]
                    .rearrange("p (kh kw co) -> p kh kw co", kh=3, kw=3)[
                        :, :, :, :CMID
                    ],
                    w1rt[i].rearrange("p (kh kw co) -> p kh kw co", kh=3, kw=3),
                )
            nc.vector.memset(w2pack, 0.0)
            nc.vector.tensor_copy(
                w2pack.rearrange("p (kh kw co) -> p kh kw co", kh=3, kw=3)[
                    :, :, :, :CMID
                ],
                w2s.rearrange("p (kh kw) -> p kh kw", kh=3)
                .unsqueeze(3)
                .broadcast_to([CMID, 3, 3, CMID]),
            )
            emit_casts()
            for i in range(2):
                # wmod[ci, k, co] = weightT[ci, k, co] * A_w[ci, k]  (cast to bf16)
                nc.vector.tensor_mul(
                    wmod[i].rearrange("p (k co) -> p k co", co=256),
                    wrt[i].rearrange("p (k co) -> p k co", co=256),
                    aw[i].unsqueeze(2).broadcast_to([128, 9, 256]),
                )

            mid_v = mid[:, 1 : 1 + HWP].rearrange("p (h c) -> p h c", c=WP)
            TFv = RT * WP
            wmod_v = [wmod[i].rearrange("p (k co) -> p k co", co=256) for i in range(2)]

            # -------- conv group emitters --------
            # SE convs: the 3 kw taps are packed into the stationary columns
            # (48 = 3 kw x 16 ch), then reduced across partition groups with
            # +-1-shifted identity matmuls. Junk in pad columns only.
            def conv1_pack(t):
                r0 = t * RT
                mps = ps.tile([96, TFv], F32, name="mps96", tag="pack", bufs=2)
                n_mm = 0
                for i in range(2):
                    for dh in (0, -1, 1):
                        kh = dh + 1
                        rl, rh = _rows(r0, dh)
                        n_mm += 1
                        nc.tensor.matmul(
                            mps[:, rl * WP : rh * WP],
                            w1pack[i][:, kh * 96 : (kh + 1) * 96],
                            xs[i][:, 1 + (r0 + rl + dh) * WP :][:128, : (rh - rl) * WP],
                            start=(n_mm == 1),
                            stop=(n_mm == 6),
                        )
                u = u1pp[t % 2]
                # drain each kw strip with its +-1 column shift baked in, so
                # one K=96 selector matmul can reduce without further shifts
                ident = mybir.ActivationFunctionType.Identity
                nc.vector.tensor_copy(u[0:16, 1:TFv], mps[0:16, 0 : TFv - 1])
                nc.scalar.activation(u[32:48, :], mps[32:48, :], ident)
                nc.scalar.activation(u[64:80, 0 : TFv - 1], mps[64:80, 1:TFv], ident)
                return u

            def conv1_sel(t, u):
                r0 = t * RT
                mid_ps = ps.tile([CMID, TFv], F32, name="mid_ps", tag="red", bufs=2)
                nc.tensor.matmul(mid_ps, identE, u, start=True, stop=True)
                mpv = mid_ps.rearrange("p (h c) -> p h c", c=WP)
                nc.scalar.activation(
                    mid_v[:, r0 : r0 + RT, 1 : W + 1],
                    mpv[:, :, 1 : W + 1],
                    mybir.ActivationFunctionType.Relu,
                )

            def conv2_pack(t):
                r0 = t * RT
                ups = ps.tile([96, TFv], F32, name="u2ps", tag="pack", bufs=2)
                n_mm = 0
                for dh in (0, -1, 1):
                    kh = dh + 1
                    rl, rh = _rows(r0, dh)
                    n_mm += 1
                    nc.tensor.matmul(
                        ups[:, rl * WP : rh * WP],
                        w2pack[:, kh * 96 : (kh + 1) * 96],
                        mid[:, 1 + (r0 + rl + dh) * WP :][:CMID, : (rh - rl) * WP],
                        start=(n_mm == 1),
                        stop=(n_mm == 3),
                    )
                u = u2pp[t % 2]
                ident = mybir.ActivationFunctionType.Identity
                nc.vector.tensor_copy(u[0:16, 1:TFv], ups[0:16, 0 : TFv - 1])
                nc.scalar.activation(u[32:48, :], ups[32:48, :], ident)
                nc.scalar.activation(u[64:80, 0 : TFv - 1], ups[64:80, 1:TFv], ident)
                return u

            def conv2_sel(t, u):
                r0 = t * RT
                aps = ps.tile([128, TFv], F32, name="aps", tag="red", bufs=2)
                nc.tensor.matmul(aps, identTE, u, start=True, stop=True)
                nc.scalar.activation(
                    asb[:, r0 * WP : (r0 + RT) * WP],
                    aps,
                    mybir.ActivationFunctionType.Sigmoid,
                )

            def main_group(t, c, fused):
                r0 = t * RT
                yps = ps.tile([128, RT * WP], F32, name="yps", tag="yps", bufs=4)
                n_mm = 0
                for i in range(2):
                    for dh, dw in TAPS:
                        k = (dh + 1) * 3 + (dw + 1)
                        rl, rh = _rows(r0, dh)
                        n_mm += 1
                        nc.tensor.matmul(
                            yps[:, rl * WP : rh * WP],
                            wmod_v[i][:, k, c * 128 : (c + 1) * 128],
                            xs[i][:, 1 + (r0 + rl + dh) * WP + dw :][:128, : (rh - rl) * WP],
                            start=(n_mm == 1),
                            stop=(n_mm == 18),
                        )
                dst = osb[c][:, r0 * WP : (r0 + RT) * WP]
                if fused:
                    nc.vector.tensor_mul(dst, yps, asb[:, r0 * WP : (r0 + RT) * WP])
                    ov = osb[c].rearrange("p (h c) -> p h c", c=WP)
                    q = nc.sync if (t + c) % 2 == 0 else nc.scalar
                    q.dma_start(
                        out_v[c][:, r0 : r0 + RT, :], ov[:, r0 : r0 + RT, 1 : W + 1]
                    )
                else:
                    nc.vector.tensor_copy(dst, yps)

            # -------- interleaved schedule --------
            # main groups in issue order; SE groups threaded between them so
            # the PE never sees a long run of thin (16-wide) matmuls.
            main_q = [(t, c) for t in range(NT) for c in range(2)]
            mq = iter(main_q)
            deferred = []
            sig_done = [False] * NT

            def emit_main(n, fused_allowed):
                for _ in range(n):
                    tc_ = next(mq, None)
                    if tc_ is None:
                        return
                    t, c = tc_
                    if sig_done[t] and fused_allowed:
                        main_group(t, c, fused=True)
                    else:
                        main_group(t, c, fused=False)
                        deferred.append((t, c))

            def flush_deferred():
                rest = []
                for t, c in deferred:
                    if not sig_done[t]:
                        rest.append((t, c))
                        continue
                    r0 = t * RT
                    dst = osb[c][:, r0 * WP : (r0 + RT) * WP]
                    nc.vector.tensor_mul(dst, dst, asb[:, r0 * WP : (r0 + RT) * WP])
                    ov = osb[c].rearrange("p (h c) -> p h c", c=WP)
                    q = nc.sync if (t + c) % 2 == 0 else nc.scalar
                    q.dma_start(
                        out_v[c][:, r0 : r0 + RT, :], ov[:, r0 : r0 + RT, 1 : W + 1]
                    )
                deferred[:] = rest

            u_prev = None
            for t in range(NT):
                u = conv1_pack(t)
                if u_prev is not None:
                    conv1_sel(t - 1, u_prev)
                    emit_main(1, fused_allowed=False)
                u_prev = u
            conv1_sel(NT - 1, u_prev)
            emit_main(1, fused_allowed=False)
            u_prev = None
            for t in range(NT):
                u = conv2_pack(t)
                if u_prev is not None:
                    conv2_sel(t - 1, u_prev)
                    sig_done[t - 1] = True
                    if t % 2 == 1:
                        emit_main(1, fused_allowed=True)
                        flush_deferred()
                u_prev = u
            conv2_sel(NT - 1, u_prev)
            sig_done[NT - 1] = True
            # remaining main groups: `a` is fully available, fuse the multiply
            emit_main(len(main_q), fused_allowed=True)
            flush_deferred()

    nc.compile()
    return nc


_NC = None


def make_in_maps(x, weight, A_w, se_w1, se_w2):
    x = np.ascontiguousarray(np.asarray(x, dtype=np.float32))
    weightT = np.ascontiguousarray(
        np.asarray(weight, dtype=np.float32).transpose(1, 2, 3, 0)
    )
    A_w = np.ascontiguousarray(np.asarray(A_w, dtype=np.float32))
    se_w1T = np.ascontiguousarray(
        np.asarray(se_w1, dtype=np.float32).transpose(1, 2, 3, 0)
    )
    se_w2 = np.ascontiguousarray(np.asarray(se_w2, dtype=np.float32))

    in_maps = [
        {
            "x": np.ascontiguousarray(x[b]),
            "weightT": weightT,
            "A_w": A_w,
            "se_w1T": se_w1T,
            "se_w2": se_w2,
        }
        for b in range(B)
    ]
    return in_maps


def kernel(x, weight, A_w, se_w1, se_w2):
    global _NC
    if _NC is None:
        _NC = build()
    in_maps = make_in_maps(x, weight, A_w, se_w1, se_w2)
    res = run_bass_kernel_spmd(_NC, in_maps, list(range(N_CORES)))
    out = np.stack([res.results[b]["out"] for b in range(B)], axis=0)
    return out


# revision 25
# speedup vs baseline: 1.1991x; 1.0595x over previous
"""Trainium2 Bass kernel for: out = conv3x3(x, weight*A_w) * sigmoid(conv3x3(relu(conv3x3(x, se_w1)), se_w2))

Sharding: data-parallel over batch B=8 -> 8 NeuronCores (one image per core);
weight / A_w / se_w1 / se_w2 replicated to every core. The conv weights are
passed transposed to [ci, kh, kw, co] (host-side layout prep during sharding)
so the matmul stationary operand loads straight from DRAM.

Per-core kernel (direct conv as implicit GEMM on the TensorEngine):
  - x stored column-padded [ci, 56, 58] bf16 in SBUF (pad cols zeroed,
    +1-element guards at both flat ends) so every 3x3 tap is a contiguous
    1-D shifted window (the matmul ISA requires single-free-dim operands).
  - row taps at the image top/bottom use clipped row ranges; the center tap
    is issued first per ci-block pass (full coverage, start=True), the
    clipped taps accumulate -> exact zero-padding semantics.
  - A_w applied on-device as a VectorE broadcast multiply during the
    f32 -> bf16 weight cast.
  - compute dtype bf16 (fp32 PSUM accumulate), rel-err vs fp32 ~3e-3.
  - thin SE-branch matmul groups (16-wide) are interleaved with dense
    128x128 main-conv groups to keep the PE activity monitor from
    re-throttling the clock (HAM).
  - main-conv PSUM tiles drain to SBUF; the attention multiply is fused
    when `a` for that tile is already available, otherwise applied in a
    deferred VectorE pass before the output DMA.
"""

import numpy as np

import concourse.bass as bass  # noqa: F401
import concourse.mybir as mybir
import concourse.tile as tile
from concourse import bacc
from concourse.bass_utils import run_bass_kernel_spmd
from concourse.masks import make_identity

B, C, H, W = 8, 256, 56, 56
HW = H * W
WP = W + 2                      # padded row width (c=0 left pad, c=57 right pad)
HWP = H * WP                    # 3248
CMID = 16
N_CORES = 8
RT = 8                          # output rows per PSUM tile
NT = H // RT                    # 7
F32 = mybir.dt.float32
BF16 = mybir.dt.bfloat16

# center tap first within each ci-block pass
TAPS = [(0, 0)] + [
    (dh, dw) for dh in (-1, 0, 1) for dw in (-1, 0, 1) if (dh, dw) != (0, 0)
]


def _rows(r0, dh):
    """Clipped local row range [rl, rh) of a tile at base row r0 for row-tap dh."""
    return max(0, -dh - r0), min(RT, H - dh - r0)


def build():
    nc = bacc.Bacc("TRN2", target_bir_lowering=False, debug=False, num_devices=N_CORES)

    x_d = nc.dram_tensor("x", [C, H, W], BF16, kind="ExternalInput").ap()
    # transposed on host: [ci, kh, kw, co]
    wt_d = nc.dram_tensor("weightT", [C, 3, 3, C], BF16, kind="ExternalInput").ap()
    aw_d = nc.dram_tensor("A_w", [1, C, 3, 3], F32, kind="ExternalInput").ap()
    # transposed on host: [ci, kh, kw, cmid]
    w1t_d = nc.dram_tensor("se_w1T", [C, 3, 3, CMID], BF16, kind="ExternalInput").ap()
    w2_d = nc.dram_tensor("se_w2", [1, CMID, 3, 3], F32, kind="ExternalInput").ap()
    out_d = nc.dram_tensor("out", [C, H, W], F32, kind="ExternalOutput").ap()

    x_v = x_d.rearrange("(b p) h w -> b p (h w)", b=2)                  # [2,128,3136]
    wt_v = wt_d.rearrange("(b p) kh kw co -> b p (kh kw co)", b=2)      # [2,128,2304]
    aw_v = aw_d[0].rearrange("(b p) kh kw -> b p (kh kw)", b=2)         # [2,128,9]
    w1t_v = w1t_d.rearrange("(b p) kh kw co -> b p (kh kw co)", b=2)    # [2,128,144]
    w2_v = w2_d[0].rearrange("p kh kw -> p (kh kw)")                    # [16,9]
    out_v = out_d.rearrange("(b p) h w -> b p h w", b=2)                # [2,128,56,56]

    with tile.TileContext(nc) as tc:
        with (
            tc.tile_pool(name="sb", bufs=1) as sb,
            tc.tile_pool(name="ps", space="PSUM", bufs=2) as ps,
        ):
            # +2: one guard element at each flat end (dw=+-1 at image corners)
            xs = [sb.tile([128, HWP + 2], BF16, name=f"xs{i}") for i in range(2)]
            xstage = [
                [sb.tile([128, HW // 2], BF16, name=f"xstage{i}{k}") for k in range(2)]
                for i in range(2)
            ]
            wrt = [sb.tile([128, 2304], BF16, name=f"wrt{i}") for i in range(2)]
            w1rt = [sb.tile([128, 9 * CMID], BF16, name=f"w1rt{i}") for i in range(2)]
            aw = [sb.tile([128, 9], F32, name=f"aw{i}") for i in range(2)]
            w2s = sb.tile([CMID, 9], F32, name="w2s")
            wmod = [sb.tile([128, 9 * 256], BF16, name=f"wmod{i}") for i in range(2)]
            mid = sb.tile([CMID, HWP + 2], BF16, name="mid")
            identE = sb.tile([96, CMID], BF16, name="identE")
            identTE = sb.tile([96, 128], BF16, name="identTE")
            u1pp = [sb.tile([96, RT * WP], BF16, name=f"u1pp{k}") for k in range(2)]
            u2pp = [sb.tile([96, RT * WP], BF16, name=f"u2pp{k}") for k in range(2)]
            w2pack = sb.tile([CMID, 3 * 96], BF16, name="w2pack")
            w1pack = [sb.tile([128, 3 * 96], BF16, name=f"w1pack{i}") for i in range(2)]
            asb = sb.tile([128, HWP], F32, name="asb")
            osb = [sb.tile([128, HWP], F32, name=f"osb{c}") for c in range(2)]

            # -------- loads --------
            # x first (the PE's first dependency), chunked so the bf16 cast
            # pipelines behind the DMA; one ci-block per HWDGE queue.
            HHALF = H // 2
            for i in range(2):
                q = nc.scalar if i == 0 else nc.sync
                for k, h0 in enumerate((0, HHALF)):
                    q.dma_start(
                        xstage[i][k],
                        x_v[i][:, h0 * W : (h0 + HHALF) * W],
                    )
            nc.scalar.dma_start(wrt[0], wt_v[0])
            nc.sync.dma_start(wrt[1], wt_v[1])
            for i in range(2):
                nc.gpsimd.dma_start(w1rt[i], w1t_v[i])
                nc.gpsimd.dma_start(aw[i], aw_v[i])
            nc.gpsimd.dma_start(w2s, w2_v)

            # -------- x pad + cast (DVE, ahead of weight prep) --------
            for tl, np_ in ((xs[0], 128), (xs[1], 128), (mid, CMID)):
                nc.vector.memset(tl[:np_, 0:2], 0.0)
                nc.vector.memset(tl[:np_, HWP : HWP + 2], 0.0)
                pads = tl[:np_, 1 + W + 1 : 1 + W + 1 + (H - 1) * WP].rearrange(
                    "p (h c) -> p h c", c=WP
                )
                nc.vector.memset(pads[:, :, 0:2], 0.0)
            def emit_casts():
                for k, h0 in enumerate((0, HHALF)):
                    for i in range(2):
                        xsv = xs[i][:, 1 : 1 + HWP].rearrange("p (h c) -> p h c", c=WP)
                        nc.vector.tensor_copy(
                            xsv[:, h0 : h0 + HHALF, 1 : W + 1],
                            xstage[i][k].rearrange("p (h w) -> p h w", w=W),
                        )

            # -------- weight prep (VectorE only, no PE) --------
            for k in range(2):
                nc.vector.memset(u1pp[k], 0.0)
                nc.vector.memset(u2pp[k], 0.0)
            # identity selectors, one copy per 32-aligned strip (matmul
            # operands must share a 32-aligned partition base)
            nc.vector.memset(identE, 0.0)
            nc.vector.memset(identTE, 0.0)
            for g in range(3):
                make_identity(nc, identE[32 * g : 32 * g + CMID, :], nomemset=True)
                nc.vector.tensor_copy(
                    identTE[32 * g : 32 * g + CMID, :].rearrange(
                        "p (r c) -> p r c", c=CMID
                    ),
                    identE[32 * g : 32 * g + CMID, :]
                    .unsqueeze(1)
                    .broadcast_to([CMID, 8, CMID]),
                )
            # kw groups packed into stationary cols at 32-col strides
            for i in range(2):
                nc.vector.memset(w1pack[i], 0.0)
                nc.vector.tensor_copy(
                    w1pack[i]
                    .rearrange("p (kh kw co) -> p kh kw co", kh=3, kw=3)[
                        :, :, :, :CMID
                    ],
                    w1rt[i].rearrange("p (kh kw co) -> p kh kw co", kh=3, kw=3),
                )
            nc.vector.memset(w2pack, 0.0)
            nc.vector.tensor_copy(
                w2pack.rearrange("p (kh kw co) -> p kh kw co", kh=3, kw=3)[
                    :, :, :, :CMID
                ],
                w2s.rearrange("p (kh kw) -> p kh kw", kh=3)
                .unsqueeze(3)
                .broadcast_to([CMID, 3, 3, CMID]),
            )
            emit_casts()
            for i in range(2):
                # wmod[ci, k, co] = weightT[ci, k, co] * A_w[ci, k]  (cast to bf16)
                nc.vector.tensor_mul(
                    wmod[i].rearrange("p (k co) -> p k co", co=256),
                    wrt[i].rearrange("p (k co) -> p k co", co=256),
                    aw[i].unsqueeze(2).broadcast_to([128, 9, 256]),
                )

            mid_v = mid[:, 1 : 1 + HWP].rearrange("p (h c) -> p h c", c=WP)
            TFv = RT * WP
            wmod_v = [wmod[i].rearrange("p (k co) -> p k co", co=256) for i in range(2)]

            # -------- conv group emitters --------
            # SE convs: the 3 kw taps are packed into the stationary columns
            # (48 = 3 kw x 16 ch), then reduced across partition groups with
            # +-1-shifted identity matmuls. Junk in pad columns only.
            def conv1_pack(t):
                r0 = t * RT
                mps = ps.tile([96, TFv], F32, name="mps96", tag="pack", bufs=2)
                n_mm = 0
                for i in range(2):
                    for dh in (0, -1, 1):
                        kh = dh + 1
                        rl, rh = _rows(r0, dh)
                        n_mm += 1
                        nc.tensor.matmul(
                            mps[:, rl * WP : rh * WP],
                            w1pack[i][:, kh * 96 : (kh + 1) * 96],
                            xs[i][:, 1 + (r0 + rl + dh) * WP :][:128, : (rh - rl) * WP],
                            start=(n_mm == 1),
                            stop=(n_mm == 6),
                        )
                u = u1pp[t % 2]
                # drain each kw strip with its +-1 column shift baked in, so
                # one K=96 selector matmul can reduce without further shifts
                ident = mybir.ActivationFunctionType.Identity
                nc.vector.tensor_copy(u[0:16, 1:TFv], mps[0:16, 0 : TFv - 1])
                nc.scalar.activation(u[32:48, :], mps[32:48, :], ident)
                nc.scalar.activation(u[64:80, 0 : TFv - 1], mps[64:80, 1:TFv], ident)
                return u

            def conv1_sel(t, u):
                r0 = t * RT
                mid_ps = ps.tile([CMID, TFv], F32, name="mid_ps", tag="red", bufs=2)
                nc.tensor.matmul(mid_ps, identE, u, start=True, stop=True)
                mpv = mid_ps.rearrange("p (h c) -> p h c", c=WP)
                nc.scalar.activation(
                    mid_v[:, r0 : r0 + RT, 1 : W + 1],
                    mpv[:, :, 1 : W + 1],
                    mybir.ActivationFunctionType.Relu,
                )

            def conv2_pack(t):
                r0 = t * RT
                ups = ps.tile([96, TFv], F32, name="u2ps", tag="pack", bufs=2)
                n_mm = 0
                for dh in (0, -1, 1):
                    kh = dh + 1
                    rl, rh = _rows(r0, dh)
                    n_mm += 1
                    nc.tensor.matmul(
                        ups[:, rl * WP : rh * WP],
                        w2pack[:, kh * 96 : (kh + 1) * 96],
                        mid[:, 1 + (r0 + rl + dh) * WP :][:CMID, : (rh - rl) * WP],
                        start=(n_mm == 1),
                        stop=(n_mm == 3),
                    )
                u = u2pp[t % 2]
                ident = mybir.ActivationFunctionType.Identity
                nc.vector.tensor_copy(u[0:16, 1:TFv], ups[0:16, 0 : TFv - 1])
                nc.scalar.activation(u[32:48, :], ups[32:48, :], ident)
                nc.scalar.activation(u[64:80, 0 : TFv - 1], ups[64:80, 1:TFv], ident)
                return u

            def conv2_sel(t, u):
                r0 = t * RT
                aps = ps.tile([128, TFv], F32, name="aps", tag="red", bufs=2)
                nc.tensor.matmul(aps, identTE, u, start=True, stop=True)
                nc.scalar.activation(
                    asb[:, r0 * WP : (r0 + RT) * WP],
                    aps,
                    mybir.ActivationFunctionType.Sigmoid,
                )

            def main_group(t, c, fused):
                r0 = t * RT
                yps = ps.tile([128, RT * WP], F32, name="yps", tag="yps", bufs=4)
                n_mm = 0
                for i in range(2):
                    for dh, dw in TAPS:
                        k = (dh + 1) * 3 + (dw + 1)
                        rl, rh = _rows(r0, dh)
                        n_mm += 1
                        nc.tensor.matmul(
                            yps[:, rl * WP : rh * WP],
                            wmod_v[i][:, k, c * 128 : (c + 1) * 128],
                            xs[i][:, 1 + (r0 + rl + dh) * WP + dw :][:128, : (rh - rl) * WP],
                            start=(n_mm == 1),
                            stop=(n_mm == 18),
                        )
                dst = osb[c][:, r0 * WP : (r0 + RT) * WP]
                if fused:
                    nc.vector.tensor_mul(dst, yps, asb[:, r0 * WP : (r0 + RT) * WP])
                    ov = osb[c].rearrange("p (h c) -> p h c", c=WP)
                    q = nc.sync if (t + c) % 2 == 0 else nc.scalar
                    q.dma_start(
                        out_v[c][:, r0 : r0 + RT, :], ov[:, r0 : r0 + RT, 1 : W + 1]
                    )
                else:
                    nc.vector.tensor_copy(dst, yps)

            # -------- interleaved schedule --------
            # main groups in issue order; SE groups threaded between them so
            # the PE never sees a long run of thin (16-wide) matmuls.
            main_q = [(t, c) for t in range(NT) for c in range(2)]
            mq = iter(main_q)
            deferred = []
            sig_done = [False] * NT

            def emit_main(n, fused_allowed):
                for _ in range(n):
                    tc_ = next(mq, None)
                    if tc_ is None:
                        return
                    t, c = tc_
                    if sig_done[t] and fused_allowed:
                        main_group(t, c, fused=True)
                    else:
                        main_group(t, c, fused=False)
                        deferred.append((t, c))

            def flush_deferred():
                rest = []
                for t, c in deferred:
                    if not sig_done[t]:
                        rest.append((t, c))
                        continue
                    r0 = t * RT
                    dst = osb[c][:, r0 * WP : (r0 + RT) * WP]
                    nc.vector.tensor_mul(dst, dst, asb[:, r0 * WP : (r0 + RT) * WP])
                    ov = osb[c].rearrange("p (h c) -> p h c", c=WP)
                    q = nc.sync if (t + c) % 2 == 0 else nc.scalar
                    q.dma_start(
                        out_v[c][:, r0 : r0 + RT, :], ov[:, r0 : r0 + RT, 1 : W + 1]
                    )
                deferred[:] = rest

            u_prev = None
            for t in range(NT):
                u = conv1_pack(t)
                if u_prev is not None:
                    conv1_sel(t - 1, u_prev)
                    emit_main(1, fused_allowed=False)
                u_prev = u
            conv1_sel(NT - 1, u_prev)
            emit_main(1, fused_allowed=False)
            u_prev = None
            for t in range(NT):
                u = conv2_pack(t)
                if u_prev is not None:
                    conv2_sel(t - 1, u_prev)
                    sig_done[t - 1] = True
                    if t % 2 == 1:
                        emit_main(1, fused_allowed=True)
                        flush_deferred()
                u_prev = u
            conv2_sel(NT - 1, u_prev)
            sig_done[NT - 1] = True
            # remaining main groups: `a` is fully available, fuse the multiply
            emit_main(len(main_q), fused_allowed=True)
            flush_deferred()

    nc.compile()
    return nc


_NC = None


def make_in_maps(x, weight, A_w, se_w1, se_w2):
    import ml_dtypes

    bf16 = ml_dtypes.bfloat16
    x = np.ascontiguousarray(np.asarray(x, dtype=np.float32).astype(bf16))
    weightT = np.ascontiguousarray(
        np.asarray(weight, dtype=np.float32).transpose(1, 2, 3, 0).astype(bf16)
    )
    A_w = np.ascontiguousarray(np.asarray(A_w, dtype=np.float32))
    se_w1T = np.ascontiguousarray(
        np.asarray(se_w1, dtype=np.float32).transpose(1, 2, 3, 0).astype(bf16)
    )
    se_w2 = np.ascontiguousarray(np.asarray(se_w2, dtype=np.float32))

    in_maps = [
        {
            "x": np.ascontiguousarray(x[b]),
            "weightT": weightT,
            "A_w": A_w,
            "se_w1T": se_w1T,
            "se_w2": se_w2,
        }
        for b in range(B)
    ]
    return in_maps


def kernel(x, weight, A_w, se_w1, se_w2):
    global _NC
    if _NC is None:
        _NC = build()
    in_maps = make_in_maps(x, weight, A_w, se_w1, se_w2)
    res = run_bass_kernel_spmd(_NC, in_maps, list(range(N_CORES)))
    out = np.stack([res.results[b]["out"] for b in range(B)], axis=0)
    return out


# revision 26
# speedup vs baseline: 1.2097x; 1.0088x over previous
"""Trainium2 Bass kernel for: out = conv3x3(x, weight*A_w) * sigmoid(conv3x3(relu(conv3x3(x, se_w1)), se_w2))

Sharding: data-parallel over batch B=8 -> 8 NeuronCores (one image per core);
weight / A_w / se_w1 / se_w2 replicated to every core. The conv weights are
passed transposed to [ci, kh, kw, co] (host-side layout prep during sharding)
so the matmul stationary operand loads straight from DRAM.

Per-core kernel (direct conv as implicit GEMM on the TensorEngine):
  - x stored column-padded [ci, 56, 58] bf16 in SBUF (pad cols zeroed,
    +1-element guards at both flat ends) so every 3x3 tap is a contiguous
    1-D shifted window (the matmul ISA requires single-free-dim operands).
  - row taps at the image top/bottom use clipped row ranges; the center tap
    is issued first per ci-block pass (full coverage, start=True), the
    clipped taps accumulate -> exact zero-padding semantics.
  - A_w applied on-device as a VectorE broadcast multiply during the
    f32 -> bf16 weight cast.
  - compute dtype bf16 (fp32 PSUM accumulate), rel-err vs fp32 ~3e-3.
  - thin SE-branch matmul groups (16-wide) are interleaved with dense
    128x128 main-conv groups to keep the PE activity monitor from
    re-throttling the clock (HAM).
  - main-conv PSUM tiles drain to SBUF; the attention multiply is fused
    when `a` for that tile is already available, otherwise applied in a
    deferred VectorE pass before the output DMA.
"""

import numpy as np

import concourse.bass as bass  # noqa: F401
import concourse.mybir as mybir
import concourse.tile as tile
from concourse import bacc
from concourse.bass_utils import run_bass_kernel_spmd
from concourse.masks import make_identity

B, C, H, W = 8, 256, 56, 56
HW = H * W
WP = W + 2                      # padded row width (c=0 left pad, c=57 right pad)
HWP = H * WP                    # 3248
CMID = 16
N_CORES = 8
RT = 8                          # output rows per PSUM tile
NT = H // RT                    # 7
F32 = mybir.dt.float32
BF16 = mybir.dt.bfloat16

# center tap first within each ci-block pass
TAPS = [(0, 0)] + [
    (dh, dw) for dh in (-1, 0, 1) for dw in (-1, 0, 1) if (dh, dw) != (0, 0)
]


def _rows(r0, dh):
    """Clipped local row range [rl, rh) of a tile at base row r0 for row-tap dh."""
    return max(0, -dh - r0), min(RT, H - dh - r0)


def build():
    nc = bacc.Bacc("TRN2", target_bir_lowering=False, debug=False, num_devices=N_CORES)

    x_d = nc.dram_tensor("x", [C, H, W], BF16, kind="ExternalInput").ap()
    # transposed on host: [ci, kh, kw, co]
    wt_d = nc.dram_tensor("weightT", [C, 3, 3, C], BF16, kind="ExternalInput").ap()
    aw_d = nc.dram_tensor("A_w", [1, C, 3, 3], F32, kind="ExternalInput").ap()
    # transposed on host: [ci, kh, kw, cmid]
    w1t_d = nc.dram_tensor("se_w1T", [C, 3, 3, CMID], BF16, kind="ExternalInput").ap()
    w2_d = nc.dram_tensor("se_w2", [1, CMID, 3, 3], F32, kind="ExternalInput").ap()
    out_d = nc.dram_tensor("out", [C, H, W], F32, kind="ExternalOutput").ap()

    x_v = x_d.rearrange("(b p) h w -> b p (h w)", b=2)                  # [2,128,3136]
    wt_v = wt_d.rearrange("(b p) kh kw co -> b p (kh kw co)", b=2)      # [2,128,2304]
    aw_v = aw_d[0].rearrange("(b p) kh kw -> b p (kh kw)", b=2)         # [2,128,9]
    w1t_v = w1t_d.rearrange("(b p) kh kw co -> b p (kh kw co)", b=2)    # [2,128,144]
    w2_v = w2_d[0].rearrange("p kh kw -> p (kh kw)")                    # [16,9]
    out_v = out_d.rearrange("(b p) h w -> b p h w", b=2)                # [2,128,56,56]

    with tile.TileContext(nc) as tc:
        with (
            tc.tile_pool(name="sb", bufs=1) as sb,
            tc.tile_pool(name="ps", space="PSUM", bufs=2) as ps,
        ):
            # +2: one guard element at each flat end (dw=+-1 at image corners)
            xs = [sb.tile([128, HWP + 2], BF16, name=f"xs{i}") for i in range(2)]
            xstage = [
                [sb.tile([128, HW // 2], BF16, name=f"xstage{i}{k}") for k in range(2)]
                for i in range(2)
            ]
            wrt = [sb.tile([128, 2304], BF16, name=f"wrt{i}") for i in range(2)]
            w1rt = [sb.tile([128, 9 * CMID], BF16, name=f"w1rt{i}") for i in range(2)]
            aw = [sb.tile([128, 9], F32, name=f"aw{i}") for i in range(2)]
            w2s = sb.tile([CMID, 9], F32, name="w2s")
            wmod = [sb.tile([128, 9 * 256], BF16, name=f"wmod{i}") for i in range(2)]
            mid = sb.tile([CMID, HWP + 2], BF16, name="mid")
            identE = sb.tile([96, CMID], BF16, name="identE")
            identTE = sb.tile([96, 128], BF16, name="identTE")
            u1pp = [sb.tile([96, RT * WP], BF16, name=f"u1pp{k}") for k in range(2)]
            u2pp = [sb.tile([96, RT * WP], BF16, name=f"u2pp{k}") for k in range(2)]
            w2pack = sb.tile([CMID, 3 * 96], BF16, name="w2pack")
            w1pack = [sb.tile([128, 3 * 96], BF16, name=f"w1pack{i}") for i in range(2)]
            asb = sb.tile([128, HWP], F32, name="asb")
            osb = [sb.tile([128, HWP], F32, name=f"osb{c}") for c in range(2)]

            # -------- loads --------
            # x first (the PE's first dependency), chunked so the bf16 cast
            # pipelines behind the DMA; one ci-block per HWDGE queue.
            HHALF = H // 2
            for i in range(2):
                q = nc.scalar if i == 0 else nc.sync
                for k, h0 in enumerate((0, HHALF)):
                    q.dma_start(
                        xstage[i][k],
                        x_v[i][:, h0 * W : (h0 + HHALF) * W],
                    )
            nc.scalar.dma_start(w1rt[0], w1t_v[0])
            nc.sync.dma_start(w1rt[1], w1t_v[1])
            nc.scalar.dma_start(wrt[0], wt_v[0])
            nc.sync.dma_start(wrt[1], wt_v[1])
            for i in range(2):
                nc.gpsimd.dma_start(aw[i], aw_v[i])
            nc.gpsimd.dma_start(w2s, w2_v)

            # -------- x pad + cast (DVE, ahead of weight prep) --------
            def pad_memset(tl, np_):
                nc.vector.memset(tl[:np_, 0:2], 0.0)
                nc.vector.memset(tl[:np_, HWP : HWP + 2], 0.0)
                pads = tl[:np_, 1 + W + 1 : 1 + W + 1 + (H - 1) * WP].rearrange(
                    "p (h c) -> p h c", c=WP
                )
                nc.vector.memset(pads[:, :, 0:2], 0.0)

            pad_memset(xs[0], 128)
            pad_memset(xs[1], 128)
            def emit_casts():
                for k, h0 in enumerate((0, HHALF)):
                    for i in range(2):
                        xsv = xs[i][:, 1 : 1 + HWP].rearrange("p (h c) -> p h c", c=WP)
                        nc.vector.tensor_copy(
                            xsv[:, h0 : h0 + HHALF, 1 : W + 1],
                            xstage[i][k].rearrange("p (h w) -> p h w", w=W),
                        )

            # -------- weight prep (VectorE only, no PE) --------
            emit_casts()
            # kw groups packed into stationary cols at 32-col strides
            for i in range(2):
                nc.vector.memset(w1pack[i], 0.0)
                nc.vector.tensor_copy(
                    w1pack[i]
                    .rearrange("p (kh kw co) -> p kh kw co", kh=3, kw=3)[
                        :, :, :, :CMID
                    ],
                    w1rt[i].rearrange("p (kh kw co) -> p kh kw co", kh=3, kw=3),
                )
            pad_memset(mid, CMID)
            for k in range(2):
                nc.vector.memset(u1pp[k], 0.0)
                nc.vector.memset(u2pp[k], 0.0)
            # identity selectors, one copy per 32-aligned strip (matmul
            # operands must share a 32-aligned partition base)
            nc.vector.memset(identE, 0.0)
            nc.vector.memset(identTE, 0.0)
            for g in range(3):
                make_identity(nc, identE[32 * g : 32 * g + CMID, :], nomemset=True)
                nc.vector.tensor_copy(
                    identTE[32 * g : 32 * g + CMID, :].rearrange(
                        "p (r c) -> p r c", c=CMID
                    ),
                    identE[32 * g : 32 * g + CMID, :]
                    .unsqueeze(1)
                    .broadcast_to([CMID, 8, CMID]),
                )
            nc.vector.memset(w2pack, 0.0)
            nc.vector.tensor_copy(
                w2pack.rearrange("p (kh kw co) -> p kh kw co", kh=3, kw=3)[
                    :, :, :, :CMID
                ],
                w2s.rearrange("p (kh kw) -> p kh kw", kh=3)
                .unsqueeze(3)
                .broadcast_to([CMID, 3, 3, CMID]),
            )
            for i in range(2):
                # wmod[ci, k, co] = weightT[ci, k, co] * A_w[ci, k]  (cast to bf16)
                nc.vector.tensor_mul(
                    wmod[i].rearrange("p (k co) -> p k co", co=256),
                    wrt[i].rearrange("p (k co) -> p k co", co=256),
                    aw[i].unsqueeze(2).broadcast_to([128, 9, 256]),
                )

            mid_v = mid[:, 1 : 1 + HWP].rearrange("p (h c) -> p h c", c=WP)
            TFv = RT * WP
            wmod_v = [wmod[i].rearrange("p (k co) -> p k co", co=256) for i in range(2)]

            # -------- conv group emitters --------
            # SE convs: the 3 kw taps are packed into the stationary columns
            # (48 = 3 kw x 16 ch), then reduced across partition groups with
            # +-1-shifted identity matmuls. Junk in pad columns only.
            def conv1_pack(t):
                r0 = t * RT
                mps = ps.tile([96, TFv], F32, name="mps96", tag="pack", bufs=2)
                n_mm = 0
                for i in range(2):
                    for dh in (0, -1, 1):
                        kh = dh + 1
                        rl, rh = _rows(r0, dh)
                        n_mm += 1
                        nc.tensor.matmul(
                            mps[:, rl * WP : rh * WP],
                            w1pack[i][:, kh * 96 : (kh + 1) * 96],
                            xs[i][:, 1 + (r0 + rl + dh) * WP :][:128, : (rh - rl) * WP],
                            start=(n_mm == 1),
                            stop=(n_mm == 6),
                        )
                u = u1pp[t % 2]
                # drain each kw strip with its +-1 column shift baked in, so
                # one K=96 selector matmul can reduce without further shifts
                ident = mybir.ActivationFunctionType.Identity
                nc.vector.tensor_copy(u[0:16, 1:TFv], mps[0:16, 0 : TFv - 1])
                nc.scalar.activation(u[32:48, :], mps[32:48, :], ident)
                nc.scalar.activation(u[64:80, 0 : TFv - 1], mps[64:80, 1:TFv], ident)
                return u

            def conv1_sel(t, u):
                r0 = t * RT
                mid_ps = ps.tile([CMID, TFv], F32, name="mid_ps", tag="red", bufs=2)
                nc.tensor.matmul(mid_ps, identE, u, start=True, stop=True)
                mpv = mid_ps.rearrange("p (h c) -> p h c", c=WP)
                nc.scalar.activation(
                    mid_v[:, r0 : r0 + RT, 1 : W + 1],
                    mpv[:, :, 1 : W + 1],
                    mybir.ActivationFunctionType.Relu,
                )

            def conv2_pack(t):
                r0 = t * RT
                ups = ps.tile([96, TFv], F32, name="u2ps", tag="pack", bufs=2)
                n_mm = 0
                for dh in (0, -1, 1):
                    kh = dh + 1
                    rl, rh = _rows(r0, dh)
                    n_mm += 1
                    nc.tensor.matmul(
                        ups[:, rl * WP : rh * WP],
                        w2pack[:, kh * 96 : (kh + 1) * 96],
                        mid[:, 1 + (r0 + rl + dh) * WP :][:CMID, : (rh - rl) * WP],
                        start=(n_mm == 1),
                        stop=(n_mm == 3),
                    )
                u = u2pp[t % 2]
                ident = mybir.ActivationFunctionType.Identity
                nc.vector.tensor_copy(u[0:16, 1:TFv], ups[0:16, 0 : TFv - 1])
                nc.scalar.activation(u[32:48, :], ups[32:48, :], ident)
                nc.scalar.activation(u[64:80, 0 : TFv - 1], ups[64:80, 1:TFv], ident)
                return u

            def conv2_sel(t, u):
                r0 = t * RT
                aps = ps.tile([128, TFv], F32, name="aps", tag="red", bufs=2)
                nc.tensor.matmul(aps, identTE, u, start=True, stop=True)
                nc.scalar.activation(
                    asb[:, r0 * WP : (r0 + RT) * WP],
                    aps,
                    mybir.ActivationFunctionType.Sigmoid,
                )

            def main_group(t, c, fused):
                r0 = t * RT
                yps = ps.tile([128, RT * WP], F32, name="yps", tag="yps", bufs=4)
                n_mm = 0
                for i in range(2):
                    for dh, dw in TAPS:
                        k = (dh + 1) * 3 + (dw + 1)
                        rl, rh = _rows(r0, dh)
                        n_mm += 1
                        nc.tensor.matmul(
                            yps[:, rl * WP : rh * WP],
                            wmod_v[i][:, k, c * 128 : (c + 1) * 128],
                            xs[i][:, 1 + (r0 + rl + dh) * WP + dw :][:128, : (rh - rl) * WP],
                            start=(n_mm == 1),
                            stop=(n_mm == 18),
                        )
                dst = osb[c][:, r0 * WP : (r0 + RT) * WP]
                if fused:
                    nc.vector.tensor_mul(dst, yps, asb[:, r0 * WP : (r0 + RT) * WP])
                    ov = osb[c].rearrange("p (h c) -> p h c", c=WP)
                    q = nc.sync if (t + c) % 2 == 0 else nc.scalar
                    q.dma_start(
                        out_v[c][:, r0 : r0 + RT, :], ov[:, r0 : r0 + RT, 1 : W + 1]
                    )
                else:
                    nc.vector.tensor_copy(dst, yps)

            # -------- interleaved schedule --------
            # main groups in issue order; SE groups threaded between them so
            # the PE never sees a long run of thin (16-wide) matmuls.
            main_q = [(t, c) for t in range(NT) for c in range(2)]
            mq = iter(main_q)
            deferred = []
            sig_done = [False] * NT

            def emit_main(n, fused_allowed):
                for _ in range(n):
                    tc_ = next(mq, None)
                    if tc_ is None:
                        return
                    t, c = tc_
                    if sig_done[t] and fused_allowed:
                        main_group(t, c, fused=True)
                    else:
                        main_group(t, c, fused=False)
                        deferred.append((t, c))

            def flush_deferred():
                rest = []
                for t, c in deferred:
                    if not sig_done[t]:
                        rest.append((t, c))
                        continue
                    r0 = t * RT
                    dst = osb[c][:, r0 * WP : (r0 + RT) * WP]
                    nc.vector.tensor_mul(dst, dst, asb[:, r0 * WP : (r0 + RT) * WP])
                    ov = osb[c].rearrange("p (h c) -> p h c", c=WP)
                    q = nc.sync if (t + c) % 2 == 0 else nc.scalar
                    q.dma_start(
                        out_v[c][:, r0 : r0 + RT, :], ov[:, r0 : r0 + RT, 1 : W + 1]
                    )
                deferred[:] = rest

            u_prev = None
            for t in range(NT):
                u = conv1_pack(t)
                if u_prev is not None:
                    conv1_sel(t - 1, u_prev)
                    emit_main(1, fused_allowed=False)
                u_prev = u
            conv1_sel(NT - 1, u_prev)
            emit_main(1, fused_allowed=False)
            u_prev = None
            for t in range(NT):
                u = conv2_pack(t)
                if u_prev is not None:
                    conv2_sel(t - 1, u_prev)
                    sig_done[t - 1] = True
                    if t % 2 == 1:
                        emit_main(1, fused_allowed=True)
                        flush_deferred()
                u_prev = u
            conv2_sel(NT - 1, u_prev)
            sig_done[NT - 1] = True
            # remaining main groups: `a` is fully available, fuse the multiply
            emit_main(len(main_q), fused_allowed=True)
            flush_deferred()

    nc.compile()
    return nc


_NC = None


def make_in_maps(x, weight, A_w, se_w1, se_w2):
    import ml_dtypes

    bf16 = ml_dtypes.bfloat16
    x = np.ascontiguousarray(np.asarray(x, dtype=np.float32).astype(bf16))
    weightT = np.ascontiguousarray(
        np.asarray(weight, dtype=np.float32).transpose(1, 2, 3, 0).astype(bf16)
    )
    A_w = np.ascontiguousarray(np.asarray(A_w, dtype=np.float32))
    se_w1T = np.ascontiguousarray(
        np.asarray(se_w1, dtype=np.float32).transpose(1, 2, 3, 0).astype(bf16)
    )
    se_w2 = np.ascontiguousarray(np.asarray(se_w2, dtype=np.float32))

    in_maps = [
        {
            "x": np.ascontiguousarray(x[b]),
            "weightT": weightT,
            "A_w": A_w,
            "se_w1T": se_w1T,
            "se_w2": se_w2,
        }
        for b in range(B)
    ]
    return in_maps


def kernel(x, weight, A_w, se_w1, se_w2):
    global _NC
    if _NC is None:
        _NC = build()
    in_maps = make_in_maps(x, weight, A_w, se_w1, se_w2)
    res = run_bass_kernel_spmd(_NC, in_maps, list(range(N_CORES)))
    out = np.stack([res.results[b]["out"] for b in range(B)], axis=0)
    return out
